# revision 22
# baseline (speedup 1.0000x reference)
"""Trainium2 Bass kernel for single-head cross-attention.

Reference computation (B=4, Sq=Skv=2048, D=1024, fp32):
    Q = query @ Wq + bq ; K = key @ Wk + bk ; V = value @ Wv + bv
    out = softmax(Q K^T / sqrt(D)) V @ Wo + bo

Weight folding (host, exact in fp32): softmax((qWq + bq)(kWk + bk)^T) equals
softmax(q M k^T + 1 x d^T) with M = Wq Wk^T and d = (k Wk) bq, because the
per-query-row term (qWq) bk and the constant bq.bk shift every score in a row
equally and cancel in softmax. Likewise (A (vWv + bv) Wo)/sums + bo =
(A v (Wv Wo))/sums + bo2 with N = Wv Wo, bo2 = bv Wo + bo, and the A-v product
reassociated as (A v) N so BOTH attention matmuls consume raw inputs. The
device computes only:
    Q'^T[e,q] = M^T @ qT          (lhsT=M,    rhs=qT)
    S^T[kv,q] = k @ Q'^T          (lhsT=kT,   rhs=Q'^T)  raw keys, e4m3 x16
    A^T       = exp(S^T/32 + dsc) (dsc = d/32 as per-kv-partition bias)
    sums[q,1] = A @ ones          (lhsT=A^T,  rhs=ones)
    U^T[dv,q] = v^T @ A^T         (lhsT=v,    rhs=A^T)   raw values
    out[q,f]  = (U @ N) * (1/sums) + bo2    (lhsT=U^T,  rhs=N)
The scores matmul runs double-pumped fp8 (e4m3): Q' is evacuated from PSUM at
x32 into e4m3, raw keys ship at x16 in e4m3, and EXP's scale folds the 1/512
back out. Everything else is bf16 with fp32 PSUM accumulation.

Sharding: 8 shards = (batch b in 0..3) x (query half h in 0..1); core
c = 2*b + h computes output rows [h*1024,(h+1)*1024) of batch b from the full
raw key/value of that batch. No inter-core communication at all.
"""

import sys

if "/opt/trn_rl_repo" not in sys.path:
    sys.path.insert(0, "/opt/trn_rl_repo")

from contextlib import ExitStack

import ml_dtypes
import numpy as np

import concourse.bass as bass
import concourse.mybir as mybir
import concourse.tile as tile
from concourse import bacc
from concourse.bass_utils import run_bass_kernel_spmd

B, SQ, SKV, D = 4, 2048, 2048, 1024
NCORES = 8
QL = SQ // 2  # local query rows per core
P = 128
DC = D // P  # feature chunks (8)
KVC = SKV // P  # kv chunks (16)
N5 = 512
F32 = mybir.dt.float32
CDT = mybir.dt.bfloat16  # on-device compute dtype for matmul operands
F8 = mybir.dt.float8e4  # scores matmul runs double-pumped e4m3
NP_CDT = ml_dtypes.bfloat16
NP_F8 = ml_dtypes.float8_e4m3
SCALE = 1.0 / 32.0  # 1/sqrt(D)
QP8_SCALE = 32.0  # Q' stored in e4m3 at 32x (sigma ~13, max 240)
K8_SCALE = 16.0  # raw keys stored in e4m3 at 16x (sigma 16)
DR = mybir.MatmulPerfMode.DoubleRow

AF = mybir.ActivationFunctionType


def _build_tile(ctx: ExitStack, tc, aps):
    nc = tc.nc
    qT, kT, vR, m, n, dsc, bo2, out = aps

    weights = ctx.enter_context(tc.tile_pool(name="weights", bufs=1))
    big = ctx.enter_context(tc.tile_pool(name="big", bufs=1))
    attn_pool = ctx.enter_context(tc.tile_pool(name="attn", bufs=1))
    evac = ctx.enter_context(tc.tile_pool(name="evac", bufs=4))
    psum = ctx.enter_context(tc.tile_pool(name="psum", bufs=4, space="PSUM"))
    psum_s = ctx.enter_context(tc.tile_pool(name="psum_s", bufs=2, space="PSUM"))

    qT_r = qT.rearrange("(c p) n -> p c n", p=P)
    kT_r = kT.rearrange("(c p) n -> p c n", p=P)
    vR_r = vR.rearrange("(c p) d -> p c d", p=P)
    n_r = n.rearrange("(c p) e -> p c e", p=P)
    m_r = m.rearrange("(c p) e -> p c e", p=P)

    # SP ring carries the critical-path loads in consumption order (m+q for
    # Q'proj, then kS for scores, then the small bias tensors); the ACT ring,
    # idle until the first EXP, carries the late-needed bulk (raw v, n).
    # Each dma_start costs ~0.65us of sequencer issue time and a DMA waiting
    # in a ring queue blocks everything behind it, so order is everything.
    mS = weights.tile([P, DC, D], CDT, tag="mS")
    qS = weights.tile([P, DC, QL], CDT, tag="qS")
    nc.sync.dma_start(out=mS[:, 0:4, :], in_=m_r[:, 0:4, :])
    nc.sync.dma_start(out=qS[:, :, 0:N5], in_=qT_r[:, :, 0:N5])
    nc.sync.dma_start(out=mS[:, 4:8, :], in_=m_r[:, 4:8, :])
    nc.sync.dma_start(out=qS[:, :, N5:QL], in_=qT_r[:, :, N5:QL])
    kS = big.tile([P, DC, SKV], F8, tag="kS")
    nc.sync.dma_start(out=kS, in_=kT_r)
    dsc_s = weights.tile([P, KVC], F32, tag="dsc")
    nc.sync.dma_start(out=dsc_s, in_=dsc.rearrange("(c p) -> p c", p=P))
    bo2_s = weights.tile([P, D], F32, tag="bo2")
    bo2_bcast = bass.AP(tensor=bo2.tensor, offset=bo2.offset, ap=[[0, P], bo2.ap[0]])
    nc.sync.dma_start(out=bo2_s, in_=bo2_bcast)

    vS = big.tile([P, KVC, D], CDT, tag="vS")  # raw V: [kv%128, kv//128, dv]
    nS = weights.tile([P, DC, D], CDT, tag="nS")
    nc.scalar.dma_start(out=vS, in_=vR_r)
    nc.scalar.dma_start(out=nS, in_=n_r)

    ones = weights.tile([P, 1], CDT, tag="ones")
    nc.vector.memset(ones, 1.0)

    # ---- Q' projection -------------------------------------------------------
    qTo = big.tile([P, DC, QL], F8, tag="qTo")  # Q'^T: [e%128, e//128, q]
    for j in range(QL // N5):
        x_in = qS[:, :, j * N5 : (j + 1) * N5]
        for ec in range(DC):
            ps = psum.tile([P, N5], F32, tag="mm")
            for dc in range(DC):
                nc.tensor.matmul(
                    ps,
                    lhsT=mS[:, dc, ec * P : (ec + 1) * P],
                    rhs=x_in[:, dc, :],
                    start=(dc == 0),
                    stop=(dc == DC - 1),
                )
            nc.scalar.activation(
                out=qTo[:, ec, j * N5 : (j + 1) * N5],
                in_=ps,
                func=AF.Identity,
                scale=QP8_SCALE,
            )

    # ---- attention, per 512-query block --------------------------------------
    for qb in range(QL // N5):
        # scores^T -> exp (double-pumped e4m3, 256-deep reduction tiles)
        attnT = attn_pool.tile([P, KVC, N5], CDT, tag="attnT")
        for c in range(KVC):
            ps = psum.tile([P, N5], F32, tag="mm")
            for ep in range(DC // 2):
                nc.tensor.matmul(
                    ps,
                    lhsT=kS[:, 2 * ep : 2 * ep + 2, c * P : (c + 1) * P],
                    rhs=qTo[:, 2 * ep : 2 * ep + 2, qb * N5 : (qb + 1) * N5],
                    start=(ep == 0),
                    stop=(ep == DC // 2 - 1),
                    perf_mode=DR,
                )
            nc.scalar.activation(
                out=attnT[:, c, :],
                in_=ps,
                func=AF.Exp,
                bias=dsc_s[:, c : c + 1],
                scale=SCALE / (QP8_SCALE * K8_SCALE),
            )

        # softmax denominators: sums[q,1] = A^T.T @ ones, accumulated over kv
        ps_sum = psum_s.tile([P, N5 // P], F32, tag="sums")
        for s in range(N5 // P):
            for c in range(KVC):
                nc.tensor.matmul(
                    ps_sum[:, s : s + 1],
                    lhsT=attnT[:, c, s * P : (s + 1) * P],
                    rhs=ones[:, :1],
                    start=(c == 0),
                    stop=(c == KVC - 1),
                )
        r_s = evac.tile([P, N5 // P], F32, tag="recip")
        nc.vector.reciprocal(r_s, ps_sum)

        # U^T[dv, q] = v^T @ A^T  (raw values)
        outT = attn_pool.tile([P, DC, N5], CDT, tag="outT")
        for mc in range(DC):
            ps = psum.tile([P, N5], F32, tag="mm")
            for c in range(KVC):
                nc.tensor.matmul(
                    ps,
                    lhsT=vS[:, c, mc * P : (mc + 1) * P],
                    rhs=attnT[:, c, :],
                    start=(c == 0),
                    stop=(c == KVC - 1),
                )
            nc.vector.tensor_copy(out=outT[:, mc, :], in_=ps)

        # out[q, f] = (U @ N) * (1/sums) + bo2
        for s in range(N5 // P):
            for nf in range(D // N5):
                ps = psum.tile([P, N5], F32, tag="mm")
                for mc in range(DC):
                    nc.tensor.matmul(
                        ps,
                        lhsT=outT[:, mc, s * P : (s + 1) * P],
                        rhs=nS[:, mc, nf * N5 : (nf + 1) * N5],
                        start=(mc == 0),
                        stop=(mc == DC - 1),
                    )
                fin = evac.tile([P, N5], F32, tag="fin")
                nc.vector.scalar_tensor_tensor(
                    out=fin,
                    in0=ps,
                    scalar=r_s[:, s : s + 1],
                    in1=bo2_s[:, nf * N5 : (nf + 1) * N5],
                    op0=mybir.AluOpType.mult,
                    op1=mybir.AluOpType.add,
                )
                row0 = qb * N5 + s * P
                nc.sync.dma_start(
                    out=out[row0 : row0 + P, nf * N5 : (nf + 1) * N5], in_=fin
                )


def build_program():
    nc = bacc.Bacc(
        "TRN2", target_bir_lowering=False, debug=False, num_devices=NCORES
    )
    qT = nc.dram_tensor("qT", [D, QL], CDT, kind="ExternalInput").ap()
    kT = nc.dram_tensor("kT", [D, SKV], F8, kind="ExternalInput").ap()
    vR = nc.dram_tensor("vR", [SKV, D], CDT, kind="ExternalInput").ap()
    m = nc.dram_tensor("m", [D, D], CDT, kind="ExternalInput").ap()
    n = nc.dram_tensor("n", [D, D], CDT, kind="ExternalInput").ap()
    dsc = nc.dram_tensor("dsc", [SKV], F32, kind="ExternalInput").ap()
    bo2 = nc.dram_tensor("bo2", [D], F32, kind="ExternalInput").ap()
    out = nc.dram_tensor("out", [QL, D], F32, kind="ExternalOutput").ap()

    with tile.TileContext(nc) as tc:
        with ExitStack() as ctx:
            _build_tile(ctx, tc, (qT, kT, vR, m, n, dsc, bo2, out))
    nc.compile()
    return nc


def prep_in_maps(query, key, value, Wq, bq, Wk, bk, Wv, bv, Wo, bo):
    """Host-side shard prep: fold weights, slice, transpose where needed."""
    query = np.asarray(query, np.float32)
    key = np.asarray(key, np.float32)
    value = np.asarray(value, np.float32)
    Wq = np.asarray(Wq, np.float32)
    Wk = np.asarray(Wk, np.float32)
    Wv = np.asarray(Wv, np.float32)
    Wo = np.asarray(Wo, np.float32)
    bq = np.asarray(bq, np.float32)
    bv = np.asarray(bv, np.float32)
    bo = np.asarray(bo, np.float32)

    M = (Wq @ Wk.T).astype(NP_CDT)
    N = (Wv @ Wo).astype(NP_CDT)
    bo2 = bv @ Wo + bo
    h_vec = Wk @ bq  # per-kv score bias direction (kv-varying, survives softmax)
    shared = {"m": M, "n": N, "bo2": bo2}
    in_maps = []
    for b in range(B):
        kTb = np.ascontiguousarray(key[b].T * np.float32(K8_SCALE)).astype(NP_F8)
        vRb = value[b].astype(NP_CDT)
        dsc_b = (key[b] @ h_vec) * np.float32(SCALE)
        for h in range(2):
            qTb = np.ascontiguousarray(query[b, h * QL : (h + 1) * QL].T).astype(
                NP_CDT
            )
            in_maps.append(
                {
                    "qT": qTb,
                    "kT": kTb,
                    "vR": vRb,
                    "dsc": dsc_b,
                    **shared,
                }
            )
    return in_maps


_NC_CACHE = None


def _get_nc():
    global _NC_CACHE
    if _NC_CACHE is None:
        _NC_CACHE = build_program()
    return _NC_CACHE


def run(inputs, **run_kwargs):
    nc = _get_nc()
    in_maps = prep_in_maps(**inputs)
    res = run_bass_kernel_spmd(nc, in_maps, core_ids=list(range(NCORES)), **run_kwargs)
    out = np.empty((B, SQ, D), np.float32)
    for b in range(B):
        for h in range(2):
            out[b, h * QL : (h + 1) * QL] = res.results[2 * b + h]["out"]
    return out, res


def kernel(query, key, value, Wq, bq, Wk, bk, Wv, bv, Wo, bo):
    out, _ = run(
        dict(
            query=query, key=key, value=value, Wq=Wq, bq=bq, Wk=Wk, bk=bk,
            Wv=Wv, bv=bv, Wo=Wo, bo=bo,
        )
    )
    return out


if __name__ == "__main__":
    rng = np.random.default_rng(0)
    ins = {
        "query": rng.standard_normal((B, SQ, D), dtype=np.float32),
        "key": rng.standard_normal((B, SKV, D), dtype=np.float32),
        "value": rng.standard_normal((B, SKV, D), dtype=np.float32),
        "Wq": (rng.standard_normal((D, D), dtype=np.float32) * 0.02),
        "bq": np.zeros(D, np.float32),
        "Wk": (rng.standard_normal((D, D), dtype=np.float32) * 0.02),
        "bk": np.zeros(D, np.float32),
        "Wv": (rng.standard_normal((D, D), dtype=np.float32) * 0.02),
        "bv": np.zeros(D, np.float32),
        "Wo": (rng.standard_normal((D, D), dtype=np.float32) * 0.02),
        "bo": np.zeros(D, np.float32),
    }
    out = kernel(**ins)
    print("kernel ran, out shape", out.shape)


# revision 23
# speedup vs baseline: 1.0667x; 1.0667x over previous
"""Trainium2 Bass kernel for single-head cross-attention.

Reference computation (B=4, Sq=Skv=2048, D=1024, fp32):
    Q = query @ Wq + bq ; K = key @ Wk + bk ; V = value @ Wv + bv
    out = softmax(Q K^T / sqrt(D)) V @ Wo + bo

Weight folding (host, exact in fp32): softmax((qWq + bq)(kWk + bk)^T) equals
softmax(q M k^T + 1 x d^T) with M = Wq Wk^T and d = (k Wk) bq, because the
per-query-row term (qWq) bk and the constant bq.bk shift every score in a row
equally and cancel in softmax. Likewise (A (vWv + bv) Wo)/sums + bo =
(A v (Wv Wo))/sums + bo2 with N = Wv Wo, bo2 = bv Wo + bo, and the A-v product
reassociated as (A v) N so BOTH attention matmuls consume raw inputs. The
device computes only:
    Q'^T[e,q] = M^T @ qT          (lhsT=M,    rhs=qT)
    S^T[kv,q] = k @ Q'^T          (lhsT=kT,   rhs=Q'^T)  raw keys, e4m3 x16
    A^T       = exp(S^T/32 + dsc) (dsc = d/32 as per-kv-partition bias)
    sums[q,1] = A @ ones          (lhsT=A^T,  rhs=ones)
    U^T[dv,q] = v^T @ A^T         (lhsT=v,    rhs=A^T)   raw values
    out[q,f]  = (U @ N) * (1/sums) + bo2    (lhsT=U^T,  rhs=N)
The scores matmul runs double-pumped fp8 (e4m3): Q' is evacuated from PSUM at
x32 into e4m3, raw keys ship at x16 in e4m3, and EXP's scale folds the 1/512
back out. Everything else is bf16 with fp32 PSUM accumulation.

Sharding: 8 shards = (batch b in 0..3) x (query half h in 0..1); core
c = 2*b + h computes output rows [h*1024,(h+1)*1024) of batch b from the full
raw key/value of that batch. No inter-core communication at all.
"""

import sys

if "/opt/trn_rl_repo" not in sys.path:
    sys.path.insert(0, "/opt/trn_rl_repo")

from contextlib import ExitStack

import ml_dtypes
import numpy as np

import concourse.bass as bass
import concourse.mybir as mybir
import concourse.tile as tile
from concourse import bacc
from concourse.bass_utils import run_bass_kernel_spmd

B, SQ, SKV, D = 4, 2048, 2048, 1024
NCORES = 8
QL = SQ // 2  # local query rows per core
P = 128
DC = D // P  # feature chunks (8)
KVC = SKV // P  # kv chunks (16)
N5 = 512
F32 = mybir.dt.float32
CDT = mybir.dt.bfloat16  # on-device compute dtype for matmul operands
F8 = mybir.dt.float8e4  # scores matmul runs double-pumped e4m3
NP_CDT = ml_dtypes.bfloat16
NP_F8 = ml_dtypes.float8_e4m3
SCALE = 1.0 / 32.0  # 1/sqrt(D)
QP8_SCALE = 32.0  # Q' stored in e4m3 at 32x (sigma ~13, max 240)
K8_SCALE = 16.0  # raw keys stored in e4m3 at 16x (sigma 16)
DR = mybir.MatmulPerfMode.DoubleRow

AF = mybir.ActivationFunctionType


def _build_tile(ctx: ExitStack, tc, aps):
    nc = tc.nc
    qT, kT, vR, m, n, dsc, bo2, out = aps

    weights = ctx.enter_context(tc.tile_pool(name="weights", bufs=1))
    big = ctx.enter_context(tc.tile_pool(name="big", bufs=1))
    attn_pool = ctx.enter_context(tc.tile_pool(name="attn", bufs=1))
    evac = ctx.enter_context(tc.tile_pool(name="evac", bufs=4))
    psum = ctx.enter_context(tc.tile_pool(name="psum", bufs=4, space="PSUM"))
    psum_s = ctx.enter_context(tc.tile_pool(name="psum_s", bufs=2, space="PSUM"))

    qT_r = qT.rearrange("(c p) n -> p c n", p=P)
    kT_r = kT.rearrange("(c p) n -> p c n", p=P)
    vR_r = vR.rearrange("(c p) d -> p c d", p=P)
    n_r = n.rearrange("(c p) e -> p c e", p=P)
    m_r = m.rearrange("(c p) e -> p c e", p=P)

    # SP ring carries the critical-path loads in consumption order (m+q for
    # Q'proj, then kS for scores, then the small bias tensors); the ACT ring,
    # idle until the first EXP, carries the late-needed bulk (raw v, n).
    # Each dma_start costs ~0.65us of sequencer issue time and a DMA waiting
    # in a ring queue blocks everything behind it, so order is everything.
    mS = weights.tile([P, DC, D], CDT, tag="mS")
    qS = weights.tile([P, DC, QL], CDT, tag="qS")
    nc.sync.dma_start(out=mS[:, 0:4, :], in_=m_r[:, 0:4, :])
    nc.sync.dma_start(out=qS[:, :, 0:N5], in_=qT_r[:, :, 0:N5])
    nc.sync.dma_start(out=mS[:, 4:8, :], in_=m_r[:, 4:8, :])
    nc.sync.dma_start(out=qS[:, :, N5:QL], in_=qT_r[:, :, N5:QL])
    kS = big.tile([P, DC, SKV], F8, tag="kS")
    nc.sync.dma_start(out=kS, in_=kT_r)
    dsc_s = weights.tile([P, KVC], F32, tag="dsc")
    nc.sync.dma_start(out=dsc_s, in_=dsc.rearrange("(c p) -> p c", p=P))
    bo2_s = weights.tile([P, D], F32, tag="bo2")
    bo2_bcast = bass.AP(tensor=bo2.tensor, offset=bo2.offset, ap=[[0, P], bo2.ap[0]])
    nc.sync.dma_start(out=bo2_s, in_=bo2_bcast)

    vS = big.tile([P, KVC, D], CDT, tag="vS")  # raw V: [kv%128, kv//128, dv]
    nS = weights.tile([P, DC, D], CDT, tag="nS")
    nc.sync.dma_start(out=vS, in_=vR_r)
    nc.sync.dma_start(out=nS, in_=n_r)

    ones = weights.tile([P, 1], CDT, tag="ones")
    nc.vector.memset(ones, 1.0)

    # ---- Q' projection -------------------------------------------------------
    qTo = big.tile([P, DC, QL], F8, tag="qTo")  # Q'^T: [e%128, e//128, q]
    for j in range(QL // N5):
        x_in = qS[:, :, j * N5 : (j + 1) * N5]
        for ec in range(DC):
            ps = psum.tile([P, N5], F32, tag="mm")
            for dc in range(DC):
                nc.tensor.matmul(
                    ps,
                    lhsT=mS[:, dc, ec * P : (ec + 1) * P],
                    rhs=x_in[:, dc, :],
                    start=(dc == 0),
                    stop=(dc == DC - 1),
                )
            nc.scalar.activation(
                out=qTo[:, ec, j * N5 : (j + 1) * N5],
                in_=ps,
                func=AF.Identity,
                scale=QP8_SCALE,
            )

    # ---- attention, per 512-query block --------------------------------------
    for qb in range(QL // N5):
        # scores^T -> exp (double-pumped e4m3, 256-deep reduction tiles)
        attnT = attn_pool.tile([P, KVC, N5], CDT, tag="attnT")
        for c in range(KVC):
            ps = psum.tile([P, N5], F32, tag="mm")
            for ep in range(DC // 2):
                nc.tensor.matmul(
                    ps,
                    lhsT=kS[:, 2 * ep : 2 * ep + 2, c * P : (c + 1) * P],
                    rhs=qTo[:, 2 * ep : 2 * ep + 2, qb * N5 : (qb + 1) * N5],
                    start=(ep == 0),
                    stop=(ep == DC // 2 - 1),
                    perf_mode=DR,
                )
            nc.scalar.activation(
                out=attnT[:, c, :],
                in_=ps,
                func=AF.Exp,
                bias=dsc_s[:, c : c + 1],
                scale=SCALE / (QP8_SCALE * K8_SCALE),
            )

        # softmax denominators: sums[q,1] = A^T.T @ ones, accumulated over kv
        ps_sum = psum_s.tile([P, N5 // P], F32, tag="sums")
        for s in range(N5 // P):
            for c in range(KVC):
                nc.tensor.matmul(
                    ps_sum[:, s : s + 1],
                    lhsT=attnT[:, c, s * P : (s + 1) * P],
                    rhs=ones[:, :1],
                    start=(c == 0),
                    stop=(c == KVC - 1),
                )
        r_s = evac.tile([P, N5 // P], F32, tag="recip")
        nc.vector.reciprocal(r_s, ps_sum)

        # U^T[dv, q] = v^T @ A^T  (raw values)
        outT = attn_pool.tile([P, DC, N5], CDT, tag="outT")
        for mc in range(DC):
            ps = psum.tile([P, N5], F32, tag="mm")
            for c in range(KVC):
                nc.tensor.matmul(
                    ps,
                    lhsT=vS[:, c, mc * P : (mc + 1) * P],
                    rhs=attnT[:, c, :],
                    start=(c == 0),
                    stop=(c == KVC - 1),
                )
            nc.vector.tensor_copy(out=outT[:, mc, :], in_=ps)

        # out[q, f] = (U @ N) * (1/sums) + bo2
        for s in range(N5 // P):
            for nf in range(D // N5):
                ps = psum.tile([P, N5], F32, tag="mm")
                for mc in range(DC):
                    nc.tensor.matmul(
                        ps,
                        lhsT=outT[:, mc, s * P : (s + 1) * P],
                        rhs=nS[:, mc, nf * N5 : (nf + 1) * N5],
                        start=(mc == 0),
                        stop=(mc == DC - 1),
                    )
                fin = evac.tile([P, N5], F32, tag="fin")
                nc.vector.scalar_tensor_tensor(
                    out=fin,
                    in0=ps,
                    scalar=r_s[:, s : s + 1],
                    in1=bo2_s[:, nf * N5 : (nf + 1) * N5],
                    op0=mybir.AluOpType.mult,
                    op1=mybir.AluOpType.add,
                )
                row0 = qb * N5 + s * P
                nc.sync.dma_start(
                    out=out[row0 : row0 + P, nf * N5 : (nf + 1) * N5], in_=fin
                )


def build_program():
    nc = bacc.Bacc(
        "TRN2", target_bir_lowering=False, debug=False, num_devices=NCORES
    )
    qT = nc.dram_tensor("qT", [D, QL], CDT, kind="ExternalInput").ap()
    kT = nc.dram_tensor("kT", [D, SKV], F8, kind="ExternalInput").ap()
    vR = nc.dram_tensor("vR", [SKV, D], CDT, kind="ExternalInput").ap()
    m = nc.dram_tensor("m", [D, D], CDT, kind="ExternalInput").ap()
    n = nc.dram_tensor("n", [D, D], CDT, kind="ExternalInput").ap()
    dsc = nc.dram_tensor("dsc", [SKV], F32, kind="ExternalInput").ap()
    bo2 = nc.dram_tensor("bo2", [D], F32, kind="ExternalInput").ap()
    out = nc.dram_tensor("out", [QL, D], F32, kind="ExternalOutput").ap()

    with tile.TileContext(nc) as tc:
        with ExitStack() as ctx:
            _build_tile(ctx, tc, (qT, kT, vR, m, n, dsc, bo2, out))
    nc.compile()
    return nc


def prep_in_maps(query, key, value, Wq, bq, Wk, bk, Wv, bv, Wo, bo):
    """Host-side shard prep: fold weights, slice, transpose where needed."""
    query = np.asarray(query, np.float32)
    key = np.asarray(key, np.float32)
    value = np.asarray(value, np.float32)
    Wq = np.asarray(Wq, np.float32)
    Wk = np.asarray(Wk, np.float32)
    Wv = np.asarray(Wv, np.float32)
    Wo = np.asarray(Wo, np.float32)
    bq = np.asarray(bq, np.float32)
    bv = np.asarray(bv, np.float32)
    bo = np.asarray(bo, np.float32)

    M = (Wq @ Wk.T).astype(NP_CDT)
    N = (Wv @ Wo).astype(NP_CDT)
    bo2 = bv @ Wo + bo
    h_vec = Wk @ bq  # per-kv score bias direction (kv-varying, survives softmax)
    shared = {"m": M, "n": N, "bo2": bo2}
    in_maps = []
    for b in range(B):
        kTb = np.ascontiguousarray(key[b].T * np.float32(K8_SCALE)).astype(NP_F8)
        vRb = value[b].astype(NP_CDT)
        dsc_b = (key[b] @ h_vec) * np.float32(SCALE)
        for h in range(2):
            qTb = np.ascontiguousarray(query[b, h * QL : (h + 1) * QL].T).astype(
                NP_CDT
            )
            in_maps.append(
                {
                    "qT": qTb,
                    "kT": kTb,
                    "vR": vRb,
                    "dsc": dsc_b,
                    **shared,
                }
            )
    return in_maps


_NC_CACHE = None


def _get_nc():
    global _NC_CACHE
    if _NC_CACHE is None:
        _NC_CACHE = build_program()
    return _NC_CACHE


def run(inputs, **run_kwargs):
    nc = _get_nc()
    in_maps = prep_in_maps(**inputs)
    res = run_bass_kernel_spmd(nc, in_maps, core_ids=list(range(NCORES)), **run_kwargs)
    out = np.empty((B, SQ, D), np.float32)
    for b in range(B):
        for h in range(2):
            out[b, h * QL : (h + 1) * QL] = res.results[2 * b + h]["out"]
    return out, res


def kernel(query, key, value, Wq, bq, Wk, bk, Wv, bv, Wo, bo):
    out, _ = run(
        dict(
            query=query, key=key, value=value, Wq=Wq, bq=bq, Wk=Wk, bk=bk,
            Wv=Wv, bv=bv, Wo=Wo, bo=bo,
        )
    )
    return out


if __name__ == "__main__":
    rng = np.random.default_rng(0)
    ins = {
        "query": rng.standard_normal((B, SQ, D), dtype=np.float32),
        "key": rng.standard_normal((B, SKV, D), dtype=np.float32),
        "value": rng.standard_normal((B, SKV, D), dtype=np.float32),
        "Wq": (rng.standard_normal((D, D), dtype=np.float32) * 0.02),
        "bq": np.zeros(D, np.float32),
        "Wk": (rng.standard_normal((D, D), dtype=np.float32) * 0.02),
        "bk": np.zeros(D, np.float32),
        "Wv": (rng.standard_normal((D, D), dtype=np.float32) * 0.02),
        "bv": np.zeros(D, np.float32),
        "Wo": (rng.standard_normal((D, D), dtype=np.float32) * 0.02),
        "bo": np.zeros(D, np.float32),
    }
    out = kernel(**ins)
    print("kernel ran, out shape", out.shape)
